# revision 14
# baseline (speedup 1.0000x reference)
"""Sparse (strided) attention Trainium2 Bass kernel, SPMD over 8 NeuronCores.

Problem: GPT-style attention block with a strided sparse mask
(STRIDE=128, C=8): each query sees its own 128-block (causal) plus the
last 8 columns of every preceding 128-block.

Sharding: batch (2) x head-groups (4) = 8 cores. Core c handles batch
c//4 and heads 4*(c%4) .. 4*(c%4)+3. Host transposes the input once per
batch, slices the weights per head group, and sums the 4 partial c_proj
outputs per batch (the tensor-parallel all-reduce) before adding b_proj.

Per-core device program (fp32):
  qkT [512,2048] = Wqk.T @ XT          (q,k head-dim-major; no transposes)
  vaug [2048,260] = X @ Wv_ext          (v seq-major; a ones column is
                                         interleaved per head via the bias
                                         trick -> softmax denominators fall
                                         out of the PV matmul for free)
  attention in S^T = [keys, queries] layout:
     S^T_local  = K_blk^T.T @ Q_blk     (PE, per 128-query block)
     S^T_summary= Ksum^T.T  @ Q_blk     (summary keys j%128>=120 of earlier blocks)
     P^T = exp(0.125*S^T)               (ScalarE; no max-subtraction: scores are
                                         O(1) because w_attn ~ N(0, 0.02^2))
     P^T_local *= uppertri_mask         (DVE, constant tile)
     hT_aug[65,q] = Vaug.T @ P^T        (PE; row 64 = softmax denominator)
     hT = hT_aug[:64] * (1/denom)       (DVE reciprocal + GPSIMD partition
                                         broadcast + DVE multiply)
  out_partial [2048,1024] = hT.T @ Wp_slice  (PE)
"""

import numpy as np

import concourse.bass as bass  # noqa: F401  (engine types pulled via nc)
import concourse.mybir as mybir
import concourse.tile as tile
from concourse import bacc
from concourse.bass_utils import run_bass_kernel_spmd

F32 = mybir.dt.float32

# float32r runs the PE at 1 cycle/row (vs 4 for float32) for moving dims
# >= 256, at ~1.4e-4 relative error (HW-measured, K=1024). The BIR verifier
# requires fp32r matmul operands to be *produced* as fp32r, so every tensor
# feeding a matmul is declared with MMDT. Set False for full-precision fp32.
USE_FP32R = True
MMDT = mybir.dt.float32r if USE_FP32R else F32

SEQ = 2048
EMB = 1024
NHEAD = 16
D = 64
STRIDE = 128
C = 8
BATCH = 2
NCORES = 8

NHL = 4                # heads per core
HD = NHL * D           # 256: head dims per core
NB = SEQ // STRIDE     # 16 query/key blocks
NG = 4                 # groups of 4 query blocks
VA = D + 1             # 65: v columns + ones column per head
VAW = NHL * VA         # 260: width of the augmented v tensor
SCALE = 1.0 / float(np.sqrt(D))  # 0.125

_CACHED_NC = None


def _emit(nc):
    xt_d = nc.dram_tensor("xt", [EMB, SEQ], MMDT, kind="ExternalInput").ap()
    wqk_d = nc.dram_tensor("wqk", [EMB, 2 * HD], MMDT, kind="ExternalInput").ap()
    wv_d = nc.dram_tensor("wv", [EMB, VAW], MMDT, kind="ExternalInput").ap()
    bqk_d = nc.dram_tensor("bqk", [1, 2 * HD], MMDT, kind="ExternalInput").ap()
    bv_d = nc.dram_tensor("bv", [1, VAW], MMDT, kind="ExternalInput").ap()
    wp_d = nc.dram_tensor("wp", [HD, EMB], MMDT, kind="ExternalInput").ap()
    maskt_d = nc.dram_tensor("maskt", [128, 512], MMDT, kind="ExternalInput").ap()
    stairs_d = nc.dram_tensor("stairs", [128, 4 * 512], MMDT,
                              kind="ExternalInput").ap()

    outp_d = nc.dram_tensor("outp", [SEQ, EMB], F32, kind="ExternalOutput").ap()
    ktd_d = nc.dram_tensor("ktd", [HD, SEQ], MMDT, kind="ExternalOutput").ap()
    vaugd_d = nc.dram_tensor("vaugd", [SEQ, VAW], MMDT, kind="ExternalOutput").ap()

    Exp = mybir.ActivationFunctionType.Exp

    with tile.TileContext(nc) as tc:
        with tc.tile_pool(name="consts", bufs=1) as consts, \
             tc.tile_pool(name="persist", bufs=1) as persist:
            # --- constants ---
            ones_row = consts.tile([1, 512], MMDT, name="ones_row", tag="ones_row")
            nc.vector.memset(ones_row, 1.0)
            zrow = consts.tile([1, 65], MMDT, name="zrow", tag="zrow")
            nc.vector.memset(zrow, 0.0)
            maskt = consts.tile([128, 512], MMDT, name="maskt", tag="maskt")
            nc.sync.dma_start(out=maskt, in_=maskt_d)
            stairs = consts.tile([128, 4 * 512], MMDT, name="stairs", tag="stairs")
            nc.sync.dma_start(out=stairs, in_=stairs_d)
            bqk = consts.tile([1, 2 * HD], MMDT, name="bqk", tag="bqk")
            nc.sync.dma_start(out=bqk, in_=bqk_d)
            bv = consts.tile([1, VAW], MMDT, name="bv", tag="bv")
            nc.sync.dma_start(out=bv, in_=bv_d)

            # --- persistent SBUF tensors ---
            wp_t = []
            for t in range(2):
                w = persist.tile([128, EMB], MMDT, name=f"wp{t}", tag=f"wp{t}")
                nc.sync.dma_start(out=w, in_=wp_d[t * 128:(t + 1) * 128, :])
                wp_t.append(w)
            qkt = [persist.tile([128, SEQ], MMDT, name=f"qkt{m}", tag=f"qkt{m}")
                   for m in range(4)]
            hT = [persist.tile([128, SEQ], MMDT, name=f"ht{t}", tag=f"ht{t}")
                  for t in range(2)]
            vaug = [persist.tile([128, VAW], MMDT, name=f"vaug{s}", tag=f"vaug{s}")
                    for s in range(NB)]
            vaugsum = persist.tile([128, VAW], MMDT, name="vaugsum", tag="vaugsum")
            ktsum = [persist.tile([128, 128], MMDT, name=f"ktsum{i}", tag=f"ktsum{i}")
                     for i in range(2)]

            # ============ phase 1: projections ============
            with tc.tile_pool(name="inp", bufs=1) as inp:
                xt_t = []
                for t in range(8):
                    x = inp.tile([128, SEQ], MMDT, name=f"xtt{t}", tag=f"xtt{t}")
                    nc.sync.dma_start(out=x, in_=xt_d[t * 128:(t + 1) * 128, :])
                    xt_t.append(x)
                wqk_t = []
                for t in range(8):
                    w = inp.tile([128, 2 * HD], MMDT, name=f"wqkt{t}", tag=f"wqkt{t}")
                    nc.sync.dma_start(out=w, in_=wqk_d[t * 128:(t + 1) * 128, :])
                    wqk_t.append(w)
                wv_t = []
                for t in range(8):
                    w = inp.tile([128, VAW], MMDT, name=f"wvt{t}", tag=f"wvt{t}")
                    nc.sync.dma_start(out=w, in_=wv_d[t * 128:(t + 1) * 128, :])
                    wv_t.append(w)

                with tc.tile_pool(name="ps1", bufs=4, space="PSUM") as ps1:
                    # qkT[m*128:(m+1)*128, n*512:(n+1)*512]
                    for m in range(4):
                        for n in range(4):
                            ps = ps1.tile([128, 512], F32, name="ps_a", tag="ps")
                            for t in range(8):
                                nc.tensor.matmul(
                                    ps,
                                    wqk_t[t][:, m * 128:(m + 1) * 128],
                                    xt_t[t][:, n * 512:(n + 1) * 512],
                                    start=(t == 0), stop=False)
                            nc.tensor.matmul(
                                ps, bqk[0:1, m * 128:(m + 1) * 128],
                                ones_row[0:1, 0:512], start=False, stop=True)
                            cp = nc.scalar.copy if (m + n) % 2 == 0 \
                                else nc.vector.tensor_copy
                            cp(out=qkt[m][:, n * 512:(n + 1) * 512], in_=ps)
                    # kT (d-major) straight out to DRAM; host transposes
                    nc.sync.dma_start(out=ktd_d[0:128, :], in_=qkt[2])
                    nc.sync.dma_start(out=ktd_d[128:256, :], in_=qkt[3])

                    # v (seq-major, ones-augmented)
                    for s in range(NB):
                        ps = ps1.tile([128, VAW], F32, name="ps_b", tag="ps")
                        for t in range(8):
                            nc.tensor.matmul(
                                ps, xt_t[t][:, s * 128:(s + 1) * 128],
                                wv_t[t], start=(t == 0), stop=False)
                        nc.tensor.matmul(
                            ps, ones_row[0:1, 0:128], bv,
                            start=False, stop=True)
                        cp = nc.scalar.copy if s % 2 == 0 else nc.vector.tensor_copy
                        cp(out=vaug[s], in_=ps)
                        nc.sync.dma_start(
                            out=vaugd_d[s * 128:(s + 1) * 128, :], in_=vaug[s])
                        # summary rows (keys j with j%128 >= 120)
                        nc.sync.dma_start(
                            out=vaugsum[s * 8:(s + 1) * 8, :],
                            in_=vaug[s][120:128, :])

            # summary key columns of kT, gathered: column 8*b+c <-> key 128*b+120+c
            for i in range(2):
                src = qkt[2 + i].rearrange("p (b s) -> p b s", s=128)[:, :, 120:128]
                dst = ktsum[i].rearrange("p (b c) -> p b c", c=8)
                nc.vector.tensor_copy(out=dst, in_=src)

            # ============ phase 2: block-sparse attention (S^T layout) ============
            with tc.tile_pool(name="psl", bufs=2, space="PSUM") as psl, \
                 tc.tile_pool(name="pss", bufs=2, space="PSUM") as pss, \
                 tc.tile_pool(name="psh", bufs=2, space="PSUM") as psh, \
                 tc.tile_pool(name="work", bufs=3) as work, \
                 tc.tile_pool(name="small", bufs=3) as small:
                for h in range(NHL):
                    ti, po = h // 2, (h % 2) * 64
                    qh = qkt[ti][po:po + 64, :]
                    kh = qkt[2 + ti][po:po + 64, :]
                    ksh = ktsum[ti][po:po + 64, :]
                    for g in range(NG):
                        nmax = 8 * (4 * g + 3)
                        gl = slice(g * 512, (g + 1) * 512)
                        ps_loc = psl.tile([128, 512], F32, name="ps_loc", tag="psloc")
                        ps_sum = pss.tile([128, 512], F32, name="ps_sum", tag="pssum")
                        for j in range(4):
                            b = 4 * g + j
                            sl = slice(j * 128, (j + 1) * 128)
                            bl = slice(b * 128, (b + 1) * 128)
                            nc.tensor.matmul(
                                ps_loc[:, sl], kh[:, bl], qh[:, bl],
                                start=True, stop=True)
                        # summary scores for all 4 query blocks in one matmul;
                        # non-causal entries are zeroed by the staircase mask
                        nc.tensor.matmul(
                            ps_sum[0:nmax, :], ksh[:, 0:nmax], qh[:, gl],
                            start=True, stop=True)
                        pt_loc = work.tile([128, 512], MMDT, name="pt_loc", tag="ptloc")
                        nc.scalar.activation(out=pt_loc, in_=ps_loc, func=Exp,
                                             scale=SCALE)
                        nc.vector.tensor_mul(out=pt_loc, in0=pt_loc, in1=maskt)
                        pt_sum = work.tile([128, 512], MMDT, name="pt_sum", tag="ptsum")
                        nc.scalar.activation(out=pt_sum[0:nmax, :],
                                             in_=ps_sum[0:nmax, :], func=Exp,
                                             scale=SCALE)
                        nc.vector.tensor_mul(out=pt_sum[0:nmax, :],
                                             in0=pt_sum[0:nmax, :],
                                             in1=stairs[0:nmax, gl])
                        ps_h = psh.tile([128, 512], F32, name="ps_h", tag="psh")
                        # zeroing matmul (0 x ones) covering every element the
                        # PV matmuls below touch, so their accumulation is
                        # order-independent regardless of scheduling
                        nc.tensor.matmul(
                            ps_h[0:65, 0:512], zrow, ones_row[0:1, 0:512],
                            start=True, stop=False, skip_group_check=True)
                        for j in range(4):
                            b = 4 * g + j
                            sl = slice(j * 128, (j + 1) * 128)
                            nc.tensor.matmul(
                                ps_h[0:65, sl], vaug[b][:, h * VA:(h + 1) * VA],
                                pt_loc[:, sl],
                                start=False, stop=False, skip_group_check=True)
                        # summary PV for all 4 blocks in one matmul (staircase
                        # mask already zeroed the invalid key rows)
                        nc.tensor.matmul(
                            ps_h[0:65, :],
                            vaugsum[0:nmax, h * VA:(h + 1) * VA],
                            pt_sum[0:nmax, :],
                            start=False, stop=True, skip_group_check=True)
                        recip = small.tile([1, 512], F32, name="recip", tag="recip")
                        nc.vector.reciprocal(out=recip, in_=ps_h[64:65, :])
                        bc = small.tile([64, 512], F32, name="bc", tag="bc")
                        nc.gpsimd.partition_broadcast(out_ap=bc, in_ap=recip,
                                                      channels=64)
                        nc.vector.tensor_mul(
                            out=hT[ti][po:po + 64, g * 512:(g + 1) * 512],
                            in0=ps_h[0:64, :], in1=bc)

            # ============ phase 3: output projection (partial) ============
            with tc.tile_pool(name="ps3", bufs=4, space="PSUM") as ps3, \
                 tc.tile_pool(name="osb", bufs=4) as osb:
                for s in range(NB):
                    for n in range(2):
                        ps = ps3.tile([128, 512], F32, name="ps_o", tag="ps3")
                        for t in range(2):
                            nc.tensor.matmul(
                                ps, hT[t][:, s * 128:(s + 1) * 128],
                                wp_t[t][:, n * 512:(n + 1) * 512],
                                start=(t == 0), stop=(t == 1))
                        ob = osb.tile([128, 512], F32, name="ob", tag="osb")
                        cp = nc.scalar.copy if (s + n) % 2 == 0 \
                            else nc.vector.tensor_copy
                        cp(out=ob, in_=ps)
                        nc.sync.dma_start(
                            out=outp_d[s * 128:(s + 1) * 128,
                                       n * 512:(n + 1) * 512], in_=ob)
    return nc


def get_nc():
    global _CACHED_NC
    if _CACHED_NC is None:
        nc = bacc.Bacc("TRN2", target_bir_lowering=False, debug=False,
                       num_devices=NCORES)
        _emit(nc)
        nc.compile()
        _CACHED_NC = nc
    return _CACHED_NC


def make_in_maps(inputs, w_attn, b_attn, w_proj, b_proj):
    inputs = np.asarray(inputs, np.float32)
    w_attn = np.asarray(w_attn, np.float32)
    b_attn = np.asarray(b_attn, np.float32)
    w_proj = np.asarray(w_proj, np.float32)

    # upper-triangular (key <= query) mask tile, repeated for 4 query blocks
    mask1 = np.triu(np.ones((128, 128), np.float32))
    maskt = np.tile(mask1, (1, 4)).copy()
    # staircase masks: stairs[kk, g*512 + j*128 + qq] = 1 iff summary key kk
    # is causally visible to query block 4g+j (kk < 8*(4g+j))
    stairs = np.zeros((128, 4 * 512), np.float32)
    for g in range(NG):
        for j in range(4):
            stairs[0:8 * (4 * g + j), g * 512 + j * 128: g * 512 + (j + 1) * 128] = 1.0

    xts = [np.ascontiguousarray(inputs[b].T) for b in range(BATCH)]

    in_maps = []
    for c in range(NCORES):
        b, hg = c // NHL, c % NHL
        q0 = hg * HD
        wq = w_attn[:, q0:q0 + HD]
        wk = w_attn[:, EMB + q0:EMB + q0 + HD]
        wv_raw = w_attn[:, 2 * EMB + q0:2 * EMB + q0 + HD]
        wqk = np.ascontiguousarray(np.concatenate([wq, wk], axis=1))
        bqk = np.concatenate(
            [b_attn[q0:q0 + HD], b_attn[EMB + q0:EMB + q0 + HD]]
        ).reshape(1, 2 * HD).astype(np.float32)
        wv = np.zeros((EMB, VAW), np.float32)
        bv = np.zeros((1, VAW), np.float32)
        for i in range(NHL):
            wv[:, i * VA:i * VA + D] = wv_raw[:, i * D:(i + 1) * D]
            bv[0, i * VA:i * VA + D] = b_attn[2 * EMB + q0 + i * D:
                                              2 * EMB + q0 + (i + 1) * D]
            bv[0, i * VA + D] = 1.0
        wp = np.ascontiguousarray(w_proj[q0:q0 + HD, :])
        in_maps.append({
            "xt": xts[b], "wqk": wqk, "wv": wv, "bqk": bqk, "bv": bv,
            "wp": wp, "maskt": maskt, "stairs": stairs,
        })
    return in_maps


def assemble(results, b_proj):
    b_proj = np.asarray(b_proj, np.float32)
    h = np.zeros((BATCH, SEQ, EMB), np.float32)
    present = np.zeros((BATCH, 2, NHEAD, SEQ, D), np.float32)
    for c in range(NCORES):
        b, hg = c // NHL, c % NHL
        h[b] += results[c]["outp"]
        ktd = results[c]["ktd"]      # [256, 2048] head-dim-major
        vaugd = results[c]["vaugd"]  # [2048, 260] with ones columns
        for i in range(NHL):
            head = hg * NHL + i
            present[b, 0, head] = ktd[i * D:(i + 1) * D, :].T
            present[b, 1, head] = vaugd[:, i * VA:i * VA + D]
    h += b_proj
    return h, present


def kernel(inputs, w_attn, b_attn, w_proj, b_proj):
    nc = get_nc()
    in_maps = make_in_maps(inputs, w_attn, b_attn, w_proj, b_proj)
    res = run_bass_kernel_spmd(nc, in_maps, core_ids=list(range(NCORES)))
    return assemble(res.results, b_proj)
